# revision 26
# baseline (speedup 1.0000x reference)
"""AttentionBlock Trainium2 kernel.

Reference computation (per batch b):
    xf = x[b].reshape(N, C);  N = 64*64 = 4096, C = 256, d = C//8 = 32
    q = xf @ Wq + bq; k = xf @ Wk + bk; v = xf @ Wv + bv
    out = softmax(q @ k.T) @ v
    y = gamma * out + xf

Sharding: 8 cores = 4 batches x 2 halves of the query rows. Each core
computes k/v for its full batch and attention for its 2048 query rows.

Per-core kernel layout choices:
  - Host passes xT (x[b] transposed, own query half rolled to the front) so
    all projection matmuls contract over channels on the partition dim.
  - q/k are projected with 4x-replicated weights (Wq tiled to [256,128]) so
    the d=32 contraction of the score matmul can be row-packed 4 ways
    (tile_position) and fill the whole 128x128 PE array.
  - Scores are computed TRANSPOSED (scoresT[m, n] = k[m].q[n]) so that after
    exp, the attention weights are already in the right layout to be the
    stationary operand of the attn@v matmul, with output in natural [n, c]
    layout - no transposes anywhere.
  - v is augmented with a ones column, so the attn@v accumulation also
    produces the softmax denominator (column 256) for free.
  - All matmuls use float32r (full-rate fp32 mode on the PE array).
"""

import numpy as np

CH = 256
DQK = 32
N = 4096  # H*W
NQ = 2048  # query rows per core
B = 4
N_CORES = 8
CH2 = CH + 2  # v augmented with [denominator-ones, pad] columns (fp32r needs even)
WBLOB = 512 + 2 * CH2  # bf16 weight blob: wq4 | wk4 | wv_aug k-tiles
CBLOB = 390  # f32 blob: bq4, bk4, row0: bv_aug, gamma, ones

_COMPILED = {}


def _build():
    """Build + compile the single-program SPMD Bass kernel. Cached."""
    if "nc" in _COMPILED:
        return _COMPILED["nc"]

    import concourse.bass as bass
    import concourse.tile as tile
    from concourse import bacc, mybir

    f32 = mybir.dt.float32
    f32r = mybir.dt.float32r
    bf16 = mybir.dt.bfloat16
    AF = mybir.ActivationFunctionType
    OP = mybir.AluOpType

    nc = bacc.Bacc(
        "TRN2",
        target_bir_lowering=False,
        debug=False,
        enable_asserts=True,
        num_devices=N_CORES,
    )

    # ---- I/O ----
    # x ships as bf16: halves the dominant (4MB) input DMA. Projections
    # compute bf16 x bf16 -> fp32 PSUM (validated rel err 1.5e-3 vs the
    # 2e-2 budget); everything downstream stays f32r.
    xT = nc.dram_tensor("xT", [CH, N], bf16, kind="ExternalInput").ap()
    xres = nc.dram_tensor("xres", [NQ, CH], f32, kind="ExternalInput").ap()
    wblob_d = nc.dram_tensor("wblob", [128, WBLOB], bf16, kind="ExternalInput").ap()
    cblob_d = nc.dram_tensor("cblob", [128, CBLOB], f32, kind="ExternalInput").ap()
    y = nc.dram_tensor("y", [NQ, CH], f32, kind="ExternalOutput").ap()

    MT = N // 128  # 32 key tiles
    NS = NQ // 512  # 4 query slices
    NGRP = MT // 4  # 8 groups of 4 key tiles

    with tile.TileContext(nc) as tc:
        with (
            tc.tile_pool(name="consts", bufs=1) as consts,
            tc.tile_pool(name="xtp", bufs=1) as xtp,
            tc.tile_pool(name="qk", bufs=1) as qkp,
            tc.tile_pool(name="vp", bufs=1) as vp,
            tc.tile_pool(name="xrp", bufs=1) as xrp,
            tc.tile_pool(name="expp", bufs=3) as expp,
            tc.tile_pool(name="yp", bufs=2) as yp,
            tc.tile_pool(name="smallp", bufs=8) as smallp,
        ):
            # ---- constants + x loads: two hw queues, FIFO order is
            # priority. wblob (gates warmup+proj) leads sync; cblob leads
            # scalar; xr strictly last so its 2MB never delays x.
            wbt = consts.tile([128, WBLOB], bf16)
            cb = consts.tile([128, CBLOB], f32r)
            nc.sync.dma_start(wbt[:], wblob_d[:, :])
            nc.scalar.dma_start(cb[:], cblob_d[:, :].bitcast(f32r))
            # views into the blobs (layout must match _pack_consts)
            wq4s = lambda kt: wbt[:, 128 * kt : 128 * (kt + 1)]
            wk4s = lambda kt: wbt[:, 256 + 128 * kt : 256 + 128 * (kt + 1)]
            wvs = lambda kt: wbt[:, 512 + CH2 * kt : 512 + CH2 * (kt + 1)]
            bq4s = cb[:, 0:1].bitcast(f32)
            bk4s = cb[:, 1:2].bitcast(f32)
            bvs = cb[0:1, 2 : 2 + CH2]
            gs = cb[0:1, 260:262]
            oness = cb[0:1, 262:390]

            xts = xtp.tile([128, 2, N], bf16)
            xTr = xT.rearrange("(t p) n -> p t n", p=128)
            for lo, hi in [(0, 1024), (1024, 2048)]:
                nc.sync.dma_start(xts[:, :, lo:hi], xTr[:, :, lo:hi])
            nc.scalar.dma_start(xts[:, :, 2048:4096], xTr[:, :, 2048:4096])

            xr = xrp.tile([128, NQ // 128, CH], f32)
            nc.scalar.dma_start(xr[:], xres.rearrange("(t p) c -> p t c", p=128))

            qt4 = qkp.tile([128, NQ], f32r)
            kt4 = qkp.tile([128, N], f32r)
            vaug = vp.tile([128, MT, CH2], f32r)

            # ---- broadcasts (bias row, gamma) via K=1 outer-product matmuls
            # plus dummy matmuls on the constant blob: they only depend on
            # the (tiny, early) cb DMA and warm the PE clock gate (HAM) so
            # the real projections run at 2.4 GHz ----
            with (
                tc.tile_pool(name="psqk", bufs=2, space="PSUM") as psqk,
                tc.tile_pool(name="psv", bufs=2, space="PSUM") as psv,
            ):
                warm_sink = consts.tile([128, 1], f32)
                for w in range(5):
                    wt = psqk.tile([128, 512], f32, tag="pqk", name=f"warm{w}")
                    nc.tensor.matmul(
                        wt[:],
                        lhsT=wbt[:, 0:128],
                        rhs=wbt[:, 0:512],
                        start=True,
                        stop=True,
                    )
                    if w == 4:
                        # keep the chain observable so it isn't dead-code
                        nc.vector.tensor_reduce(
                            warm_sink[:], wt[:], axis=mybir.AxisListType.X,
                            op=OP.max,
                        )
                # tiny exp so the ACT table set loads here (ACT is idle),
                # not right before the first real exp
                warm_exp = consts.tile([1, 2], f32)
                nc.scalar.activation(warm_exp[:], cb[0:1, 0:2].bitcast(f32), AF.Exp)
                pb = psv.tile([128, CH2], f32, tag="pv", name="pb")
                nc.tensor.matmul(
                    pb[:],
                    lhsT=oness.bitcast(f32r),
                    rhs=bvs.bitcast(f32r),
                    start=True,
                    stop=True,
                )
                bvb2 = consts.tile([128, 2, CH2], f32)
                nc.vector.tensor_copy(bvb2[:, 0, :], pb[:])
                nc.vector.tensor_copy(bvb2[:, 1, :], pb[:])

                pg = psv.tile([128, 2], f32, tag="pv", name="pg")
                nc.tensor.matmul(
                    pg[:],
                    lhsT=oness.bitcast(f32r),
                    rhs=gs.bitcast(f32r),
                    start=True,
                    stop=True,
                )
                gb = consts.tile([128, 2], f32)
                nc.vector.tensor_copy(gb[:], pg[:])

            # ---- projections (bf16 inputs -> fp32 PSUM -> f32r SBUF),
            # interleaved per 1024-col x chunk, with PAIRED evacuations:
            # one [128,1024] ACT Identity per q/k tile-pair and one
            # [128,2,258] DVE add per v tile-pair, halving the per-op
            # overhead of the evacuation chain that paces this phase ----
                def qkpair(p, ws, bias, dst):
                    pt = psqk.tile([128, 1024], f32, tag="pqk", name=f"p{p}")
                    for u in range(2):
                        for kt in range(2):
                            nc.tensor.matmul(
                                pt[:, 512 * u : 512 * (u + 1)],
                                lhsT=ws(kt),
                                rhs=xts[:, kt, 512 * (p + u) : 512 * (p + u + 1)],
                                start=(kt == 0),
                                stop=(kt == 1),
                            )
                    nc.scalar.activation(
                        dst[:, 512 * p : 512 * (p + 2)], pt[:],
                        AF.Identity, bias=bias,
                    )

                def vpair(mt):
                    pv = psv.tile([128, 2, 512], f32, tag="pv", name=f"pv{mt}")
                    for u in range(2):
                        for kt in range(2):
                            nc.tensor.matmul(
                                pv[:, u, 0:CH2],
                                lhsT=xts[:, kt, 128 * (mt + u) : 128 * (mt + u + 1)],
                                rhs=wvs(kt),
                                start=(kt == 0),
                                stop=(kt == 1),
                            )
                    nc.vector.tensor_tensor(
                        vaug[:, mt : mt + 2, :], pv[:, :, 0:CH2], bvb2[:],
                        op=OP.add,
                    )

                for p in (0, 2):  # 1024-col sections of the own query half
                    qkpair(p, wq4s, bq4s, qt4)
                    qkpair(p, wk4s, bk4s, kt4)
                    for mt in range(4 * p, 4 * p + 8, 2):
                        vpair(mt)
                for p in (4, 6):  # tail sections: k and v only
                    qkpair(p, wk4s, bk4s, kt4)
                    for mt in range(4 * p, 4 * p + 8, 2):
                        vpair(mt)

            # ---- attention main loop ----
            # Groups of 2 key tiles; score PSUM double-buffered (2+2 banks)
            # so the PE can prefill the next group's scores while the
            # ScalarE exps the current one. Consecutive groups use disjoint
            # PE row-strip pairs so their packed matmuls overlap in the
            # array as well.
            NG2 = MT // 2  # 16 groups per n-slice
            with (
                tc.tile_pool(name="pss", bufs=2, space="PSUM") as pss,
                tc.tile_pool(name="psa", bufs=1, space="PSUM") as psa,
            ):
                def scores_mm(ns, g, s):
                    # scoresT[m, n], one K=128 matmul per key tile: the 4x
                    # replication of q/k means the 128-deep contraction sums
                    # 4 copies of q.k (Wq is pre-scaled by 1/4 on the host).
                    for i in range(2):
                        mt = 2 * g + i
                        nc.tensor.matmul(
                            s[:, i, :],
                            lhsT=kt4[:, 128 * mt : 128 * (mt + 1)],
                            rhs=qt4[:, 512 * ns : 512 * (ns + 1)],
                            start=True,
                            stop=True,
                        )

                groups = [(ns, g) for ns in range(NS) for g in range(NG2)]
                acc = None
                # two groups of scores in flight ahead of the exp stream
                s_tiles = {}
                for la in range(2):
                    s_tiles[la] = pss.tile([128, 2, 512], f32, tag="s", name=f"sc{la}")
                    scores_mm(*groups[la], s_tiles[la])
                for idx, (ns, g) in enumerate(groups):
                    if g == 0:
                        acc = psa.tile([128, 4, 512], f32)
                    e = expp.tile([128, 2, 512], f32r)
                    nc.scalar.activation(e[:], s_tiles.pop(idx % 2)[:], AF.Exp)
                    # keep the scores pipeline 2 deep before emitting accums
                    if idx + 2 < len(groups):
                        s_tiles[idx % 2] = pss.tile([128, 2, 512], f32, tag="s", name=f"sc{idx}")
                        scores_mm(*groups[idx + 2], s_tiles[idx % 2])
                    # acc[n, :] += expT[m, n].T-as-weights @ v_aug[m, :]
                    for i in range(2):
                        mt = 2 * g + i
                        for j in range(4):
                            nc.tensor.matmul(
                                acc[:, j, 0:CH2],
                                lhsT=e[:, i, 128 * j : 128 * (j + 1)].bitcast(f32r),
                                rhs=vaug[:, mt, :].bitcast(f32r),
                                start=(g == 0 and i == 0),
                                stop=(g == NG2 - 1 and i == 1),
                            )
                    if g == NG2 - 1:
                        # evacuate acc quickly (one copy) so the next slice's
                        # accumulation isn't blocked on the normalize chain
                        accs = yp.tile([128, 4, CH2], f32, tag="accs")
                        for j in range(4):
                            nc.vector.tensor_copy(
                                accs[:, j, :], acc[:, j, 0:CH2]
                            )
                        yt = yp.tile([128, 4, CH], f32, tag="yt")
                        for j in range(4):
                            nt = 4 * ns + j
                            r = smallp.tile([128, 1], f32)
                            nc.vector.reciprocal(r[:], accs[:, j, CH : CH + 1])
                            rg = smallp.tile([128, 1], f32)
                            nc.vector.tensor_tensor(rg[:], r[:], gb[:, 0:1], op=OP.mult)
                            nc.vector.scalar_tensor_tensor(
                                yt[:, j, :],
                                accs[:, j, 0:CH],
                                rg[:, 0:1],
                                xr[:, nt, :],
                                op0=OP.mult,
                                op1=OP.add,
                            )
                        nc.sync.dma_start(
                            y.rearrange("(t p) c -> p t c", p=128)[
                                :, 4 * ns : 4 * (ns + 1), :
                            ],
                            yt[:],
                        )

    nc.compile()
    _COMPILED["nc"] = nc
    return nc


def _pack_consts(Wq, bq, Wk, bk, Wv, bv, gamma):
    """Pack constants into a bf16 weight blob + a small f32 blob.

    wblob [128, WBLOB] (bf16), per partition p:
      [0:256)     Wq4 k-tiles: [wq4[p], wq4[p+128]]   (wq4 = tile(Wq*0.25, (1,4)))
      [256:512)   Wk4 k-tiles
      [512:1028)  Wv_aug k-tiles (CH2 = 258 each)
    cblob [128, CBLOB] (f32):
      [0] bq4[p];  [1] bk4[p]
      partition 0 only: [2:260) bv_aug (bv ++ [1.0, 0.0]);
      [260:262) gamma, 0;  [262:390) ones
    """
    import ml_dtypes

    # Wq/bq scaled by 1/4: the K=128 score matmul sums over the 4 replicas
    Wq4 = np.tile(np.asarray(Wq, np.float32) * 0.25, (1, 4))  # [256, 128]
    Wk4 = np.tile(np.asarray(Wk, np.float32), (1, 4))
    bq4 = np.tile(np.asarray(bq, np.float32) * 0.25, 4)  # [128]
    bk4 = np.tile(np.asarray(bk, np.float32), 4)
    Wv_aug = np.zeros((CH, CH2), np.float32)
    Wv_aug[:, :CH] = np.asarray(Wv, np.float32)

    wb = np.zeros((128, WBLOB), np.float32)
    for kt in range(2):
        wb[:, 128 * kt : 128 * (kt + 1)] = Wq4[128 * kt : 128 * (kt + 1), :]
        wb[:, 256 + 128 * kt : 256 + 128 * (kt + 1)] = Wk4[128 * kt : 128 * (kt + 1)]
        wb[:, 512 + CH2 * kt : 512 + CH2 * (kt + 1)] = Wv_aug[
            128 * kt : 128 * (kt + 1), :
        ]
    cbl = np.zeros((128, CBLOB), np.float32)
    cbl[:, 0] = bq4
    cbl[:, 1] = bk4
    cbl[0, 2 : 2 + CH] = np.asarray(bv, np.float32)
    cbl[0, 2 + CH] = 1.0
    cbl[0, 260] = np.float32(np.asarray(gamma).reshape(()))
    cbl[0, 262:390] = 1.0
    return wb.astype(ml_dtypes.bfloat16), cbl


def _shard_inputs(x, Wq, bq, Wk, bk, Wv, bv, gamma):
    """Host-side prep: one input map per core."""
    import ml_dtypes

    xf = np.ascontiguousarray(x, dtype=np.float32).reshape(B, N, CH)
    wb, cbl = _pack_consts(Wq, bq, Wk, bk, Wv, bv, gamma)

    in_maps = []
    for c in range(N_CORES):
        b, h = divmod(c, 2)
        own = slice(h * NQ, (h + 1) * NQ)
        other = slice((1 - h) * NQ, (2 - h) * NQ)
        xT_b = xf[b].T  # [CH, N]
        xT_roll = np.ascontiguousarray(
            np.concatenate([xT_b[:, own], xT_b[:, other]], axis=1)
        ).astype(ml_dtypes.bfloat16)
        in_maps.append(
            {
                "xT": xT_roll,
                "xres": np.ascontiguousarray(xf[b, own]),
                "wblob": wb,
                "cblob": cbl,
            }
        )
    return in_maps


def kernel(x, Wq, bq, Wk, bk, Wv, bv, gamma):
    from concourse.bass_utils import run_bass_kernel_spmd

    nc = _build()
    in_maps = _shard_inputs(x, Wq, bq, Wk, bk, Wv, bv, gamma)
    res = run_bass_kernel_spmd(nc, in_maps, core_ids=list(range(N_CORES)))
    out = np.empty((B, N, CH), np.float32)
    for c in range(N_CORES):
        b, h = divmod(c, 2)
        out[b, h * NQ : (h + 1) * NQ, :] = res.results[c]["y"]
    return out.reshape(x.shape)



# revision 27
# speedup vs baseline: 1.0360x; 1.0360x over previous
"""AttentionBlock Trainium2 kernel.

Reference computation (per batch b):
    xf = x[b].reshape(N, C);  N = 64*64 = 4096, C = 256, d = C//8 = 32
    q = xf @ Wq + bq; k = xf @ Wk + bk; v = xf @ Wv + bv
    out = softmax(q @ k.T) @ v
    y = gamma * out + xf

Sharding: 8 cores = 4 batches x 2 halves of the query rows. Each core
computes k/v for its full batch and attention for its 2048 query rows.

Per-core kernel layout choices:
  - Host passes xT (x[b] transposed, own query half rolled to the front) so
    all projection matmuls contract over channels on the partition dim.
  - q/k are projected with 4x-replicated weights (Wq tiled to [256,128]) so
    the d=32 contraction of the score matmul can be row-packed 4 ways
    (tile_position) and fill the whole 128x128 PE array.
  - Scores are computed TRANSPOSED (scoresT[m, n] = k[m].q[n]) so that after
    exp, the attention weights are already in the right layout to be the
    stationary operand of the attn@v matmul, with output in natural [n, c]
    layout - no transposes anywhere.
  - v is augmented with a ones column, so the attn@v accumulation also
    produces the softmax denominator (column 256) for free.
  - All matmuls use float32r (full-rate fp32 mode on the PE array).
"""

import numpy as np

CH = 256
DQK = 32
N = 4096  # H*W
NQ = 2048  # query rows per core
B = 4
N_CORES = 8
CH2 = CH + 2  # v augmented with [denominator-ones, pad] columns (fp32r needs even)
WBLOB = 512 + 2 * CH2  # bf16 weight blob: wq4 | wk4 | wv_aug k-tiles
CBLOB = 390  # f32 blob: bq4, bk4, row0: bv_aug, gamma, ones

_COMPILED = {}


def _build():
    """Build + compile the single-program SPMD Bass kernel. Cached."""
    if "nc" in _COMPILED:
        return _COMPILED["nc"]

    import concourse.bass as bass
    import concourse.tile as tile
    from concourse import bacc, mybir

    f32 = mybir.dt.float32
    f32r = mybir.dt.float32r
    bf16 = mybir.dt.bfloat16
    AF = mybir.ActivationFunctionType
    OP = mybir.AluOpType

    nc = bacc.Bacc(
        "TRN2",
        target_bir_lowering=False,
        debug=False,
        enable_asserts=True,
        num_devices=N_CORES,
    )

    # ---- I/O ----
    # x ships as bf16: halves the dominant (4MB) input DMA. Projections
    # compute bf16 x bf16 -> fp32 PSUM (validated rel err 1.5e-3 vs the
    # 2e-2 budget); everything downstream stays f32r.
    xT = nc.dram_tensor("xT", [CH, N], bf16, kind="ExternalInput").ap()
    xres = nc.dram_tensor("xres", [NQ, CH], f32, kind="ExternalInput").ap()
    wblob_d = nc.dram_tensor("wblob", [128, WBLOB], bf16, kind="ExternalInput").ap()
    cblob_d = nc.dram_tensor("cblob", [128, CBLOB], f32, kind="ExternalInput").ap()
    y = nc.dram_tensor("y", [NQ, CH], f32, kind="ExternalOutput").ap()

    MT = N // 128  # 32 key tiles
    NS = NQ // 512  # 4 query slices
    NGRP = MT // 4  # 8 groups of 4 key tiles

    with tile.TileContext(nc) as tc:
        with (
            tc.tile_pool(name="consts", bufs=1) as consts,
            tc.tile_pool(name="xtp", bufs=1) as xtp,
            tc.tile_pool(name="qk", bufs=1) as qkp,
            tc.tile_pool(name="vp", bufs=1) as vp,
            tc.tile_pool(name="xrp", bufs=1) as xrp,
            tc.tile_pool(name="expp", bufs=3) as expp,
            tc.tile_pool(name="yp", bufs=2) as yp,
            tc.tile_pool(name="smallp", bufs=8) as smallp,
        ):
            # ---- constants + x loads: two hw queues, FIFO order is
            # priority. wblob (gates warmup+proj) leads sync; cblob leads
            # scalar; xr strictly last so its 2MB never delays x.
            wbt = consts.tile([128, WBLOB], bf16)
            cb = consts.tile([128, CBLOB], f32r)
            nc.sync.dma_start(wbt[:], wblob_d[:, :])
            nc.scalar.dma_start(cb[:], cblob_d[:, :].bitcast(f32r))
            # views into the blobs (layout must match _pack_consts)
            wq4s = lambda kt: wbt[:, 128 * kt : 128 * (kt + 1)]
            wk4s = lambda kt: wbt[:, 256 + 128 * kt : 256 + 128 * (kt + 1)]
            wvs = lambda kt: wbt[:, 512 + CH2 * kt : 512 + CH2 * (kt + 1)]
            bq4s = cb[:, 0:1].bitcast(f32)
            bk4s = cb[:, 1:2].bitcast(f32)
            bvs = cb[0:1, 2 : 2 + CH2]
            gs = cb[0:1, 260:262]
            oness = cb[0:1, 262:390]

            xts = xtp.tile([128, 2, N], bf16)
            xTr = xT.rearrange("(t p) n -> p t n", p=128)
            for lo, hi in [(0, 1024), (1024, 2048)]:
                nc.sync.dma_start(xts[:, :, lo:hi], xTr[:, :, lo:hi])
            nc.scalar.dma_start(xts[:, :, 2048:4096], xTr[:, :, 2048:4096])

            xr = xrp.tile([128, NQ // 128, CH], f32)
            nc.scalar.dma_start(xr[:], xres.rearrange("(t p) c -> p t c", p=128))

            qt4 = qkp.tile([128, NQ], f32r)
            kt4 = qkp.tile([128, N], f32r)
            vaug = vp.tile([128, MT, CH2], f32r)

            # ---- broadcasts (bias row, gamma) via K=1 outer-product matmuls
            # plus dummy matmuls on the constant blob: they only depend on
            # the (tiny, early) cb DMA and warm the PE clock gate (HAM) so
            # the real projections run at 2.4 GHz ----
            with (
                tc.tile_pool(name="psqk", bufs=2, space="PSUM") as psqk,
                tc.tile_pool(name="psv", bufs=2, space="PSUM") as psv,
            ):
                warm_sink = consts.tile([128, 1], f32)
                for w in range(6):
                    wt = psqk.tile([128, 512], f32, tag="pqk", name=f"warm{w}")
                    nc.tensor.matmul(
                        wt[:],
                        lhsT=wbt[:, 0:128],
                        rhs=wbt[:, 0:512],
                        start=True,
                        stop=True,
                    )
                    if w == 5:
                        # keep the chain observable so it isn't dead-code
                        nc.vector.tensor_reduce(
                            warm_sink[:], wt[:], axis=mybir.AxisListType.X,
                            op=OP.max,
                        )
                # tiny exp so the ACT table set loads here (ACT is idle),
                # not right before the first real exp
                warm_exp = consts.tile([1, 2], f32)
                nc.scalar.activation(warm_exp[:], cb[0:1, 0:2].bitcast(f32), AF.Exp)
                pb = psv.tile([128, CH2], f32, tag="pv", name="pb")
                nc.tensor.matmul(
                    pb[:],
                    lhsT=oness.bitcast(f32r),
                    rhs=bvs.bitcast(f32r),
                    start=True,
                    stop=True,
                )
                bvb2 = consts.tile([128, 2, CH2], f32)
                nc.vector.tensor_copy(bvb2[:, 0, :], pb[:])
                nc.vector.tensor_copy(bvb2[:, 1, :], pb[:])

                pg = psv.tile([128, 2], f32, tag="pv", name="pg")
                nc.tensor.matmul(
                    pg[:],
                    lhsT=oness.bitcast(f32r),
                    rhs=gs.bitcast(f32r),
                    start=True,
                    stop=True,
                )
                gb = consts.tile([128, 2], f32)
                nc.vector.tensor_copy(gb[:], pg[:])

            # ---- projections (bf16 inputs -> fp32 PSUM -> f32r SBUF),
            # interleaved per 1024-col x chunk, with PAIRED evacuations:
            # one [128,1024] ACT Identity per q/k tile-pair and one
            # [128,2,258] DVE add per v tile-pair, halving the per-op
            # overhead of the evacuation chain that paces this phase ----
                def qkpair(p, ws, bias, dst):
                    pt = psqk.tile([128, 1024], f32, tag="pqk", name=f"p{p}")
                    for u in range(2):
                        for kt in range(2):
                            nc.tensor.matmul(
                                pt[:, 512 * u : 512 * (u + 1)],
                                lhsT=ws(kt),
                                rhs=xts[:, kt, 512 * (p + u) : 512 * (p + u + 1)],
                                start=(kt == 0),
                                stop=(kt == 1),
                            )
                    nc.scalar.activation(
                        dst[:, 512 * p : 512 * (p + 2)], pt[:],
                        AF.Identity, bias=bias,
                    )

                def vpair(mt):
                    pv = psv.tile([128, 2, 512], f32, tag="pv", name=f"pv{mt}")
                    for u in range(2):
                        for kt in range(2):
                            nc.tensor.matmul(
                                pv[:, u, 0:CH2],
                                lhsT=xts[:, kt, 128 * (mt + u) : 128 * (mt + u + 1)],
                                rhs=wvs(kt),
                                start=(kt == 0),
                                stop=(kt == 1),
                            )
                    nc.vector.tensor_tensor(
                        vaug[:, mt : mt + 2, :], pv[:, :, 0:CH2], bvb2[:],
                        op=OP.add,
                    )

                for p in (0, 2):  # 1024-col sections of the own query half
                    qkpair(p, wq4s, bq4s, qt4)
                    qkpair(p, wk4s, bk4s, kt4)
                    for mt in range(4 * p, 4 * p + 8, 2):
                        vpair(mt)
                for p in (4, 6):  # tail sections: k and v only
                    qkpair(p, wk4s, bk4s, kt4)
                    for mt in range(4 * p, 4 * p + 8, 2):
                        vpair(mt)

            # ---- attention main loop ----
            # Groups of 2 key tiles; score PSUM double-buffered (2+2 banks)
            # so the PE can prefill the next group's scores while the
            # ScalarE exps the current one. Consecutive groups use disjoint
            # PE row-strip pairs so their packed matmuls overlap in the
            # array as well.
            NG2 = MT // 2  # 16 groups per n-slice
            with (
                tc.tile_pool(name="pss", bufs=2, space="PSUM") as pss,
                tc.tile_pool(name="psa", bufs=1, space="PSUM") as psa,
            ):
                def scores_mm(ns, g, s):
                    # scoresT[m, n], one K=128 matmul per key tile: the 4x
                    # replication of q/k means the 128-deep contraction sums
                    # 4 copies of q.k (Wq is pre-scaled by 1/4 on the host).
                    for i in range(2):
                        mt = 2 * g + i
                        nc.tensor.matmul(
                            s[:, i, :],
                            lhsT=kt4[:, 128 * mt : 128 * (mt + 1)],
                            rhs=qt4[:, 512 * ns : 512 * (ns + 1)],
                            start=True,
                            stop=True,
                        )

                groups = [(ns, g) for ns in range(NS) for g in range(NG2)]
                acc = None
                # two groups of scores in flight ahead of the exp stream
                s_tiles = {}
                for la in range(2):
                    s_tiles[la] = pss.tile([128, 2, 512], f32, tag="s", name=f"sc{la}")
                    scores_mm(*groups[la], s_tiles[la])
                for idx, (ns, g) in enumerate(groups):
                    if g == 0:
                        acc = psa.tile([128, 4, 512], f32)
                    e = expp.tile([128, 2, 512], f32r)
                    nc.scalar.activation(e[:], s_tiles.pop(idx % 2)[:], AF.Exp)
                    # keep the scores pipeline 2 deep before emitting accums
                    if idx + 2 < len(groups):
                        s_tiles[idx % 2] = pss.tile([128, 2, 512], f32, tag="s", name=f"sc{idx}")
                        scores_mm(*groups[idx + 2], s_tiles[idx % 2])
                    # acc[n, :] += expT[m, n].T-as-weights @ v_aug[m, :]
                    for i in range(2):
                        mt = 2 * g + i
                        for j in range(4):
                            nc.tensor.matmul(
                                acc[:, j, 0:CH2],
                                lhsT=e[:, i, 128 * j : 128 * (j + 1)].bitcast(f32r),
                                rhs=vaug[:, mt, :].bitcast(f32r),
                                start=(g == 0 and i == 0),
                                stop=(g == NG2 - 1 and i == 1),
                            )
                    if g == NG2 - 1:
                        # evacuate acc quickly (one copy) so the next slice's
                        # accumulation isn't blocked on the normalize chain
                        accs = yp.tile([128, 4, CH2], f32, tag="accs")
                        nc.vector.tensor_copy(accs[:], acc[:, :, 0:CH2])
                        yt = yp.tile([128, 4, CH], f32, tag="yt")
                        # all 4 denominators in one strided reciprocal and
                        # one broadcast multiply (fewer DVE ops = less
                        # semaphore overhead on the slice-boundary chain)
                        r4 = smallp.tile([128, 4], f32)
                        nc.vector.reciprocal(r4[:], accs[:, :, CH])
                        rg4 = smallp.tile([128, 4], f32)
                        nc.vector.tensor_scalar_mul(rg4[:], r4[:], gb[:, 0:1])
                        for j in range(4):
                            nt = 4 * ns + j
                            nc.vector.scalar_tensor_tensor(
                                yt[:, j, :],
                                accs[:, j, 0:CH],
                                rg4[:, j : j + 1],
                                xr[:, nt, :],
                                op0=OP.mult,
                                op1=OP.add,
                            )
                        nc.sync.dma_start(
                            y.rearrange("(t p) c -> p t c", p=128)[
                                :, 4 * ns : 4 * (ns + 1), :
                            ],
                            yt[:],
                        )

    nc.compile()
    _COMPILED["nc"] = nc
    return nc


def _pack_consts(Wq, bq, Wk, bk, Wv, bv, gamma):
    """Pack constants into a bf16 weight blob + a small f32 blob.

    wblob [128, WBLOB] (bf16), per partition p:
      [0:256)     Wq4 k-tiles: [wq4[p], wq4[p+128]]   (wq4 = tile(Wq*0.25, (1,4)))
      [256:512)   Wk4 k-tiles
      [512:1028)  Wv_aug k-tiles (CH2 = 258 each)
    cblob [128, CBLOB] (f32):
      [0] bq4[p];  [1] bk4[p]
      partition 0 only: [2:260) bv_aug (bv ++ [1.0, 0.0]);
      [260:262) gamma, 0;  [262:390) ones
    """
    import ml_dtypes

    # Wq/bq scaled by 1/4: the K=128 score matmul sums over the 4 replicas
    Wq4 = np.tile(np.asarray(Wq, np.float32) * 0.25, (1, 4))  # [256, 128]
    Wk4 = np.tile(np.asarray(Wk, np.float32), (1, 4))
    bq4 = np.tile(np.asarray(bq, np.float32) * 0.25, 4)  # [128]
    bk4 = np.tile(np.asarray(bk, np.float32), 4)
    Wv_aug = np.zeros((CH, CH2), np.float32)
    Wv_aug[:, :CH] = np.asarray(Wv, np.float32)

    wb = np.zeros((128, WBLOB), np.float32)
    for kt in range(2):
        wb[:, 128 * kt : 128 * (kt + 1)] = Wq4[128 * kt : 128 * (kt + 1), :]
        wb[:, 256 + 128 * kt : 256 + 128 * (kt + 1)] = Wk4[128 * kt : 128 * (kt + 1)]
        wb[:, 512 + CH2 * kt : 512 + CH2 * (kt + 1)] = Wv_aug[
            128 * kt : 128 * (kt + 1), :
        ]
    cbl = np.zeros((128, CBLOB), np.float32)
    cbl[:, 0] = bq4
    cbl[:, 1] = bk4
    cbl[0, 2 : 2 + CH] = np.asarray(bv, np.float32)
    cbl[0, 2 + CH] = 1.0
    cbl[0, 260] = np.float32(np.asarray(gamma).reshape(()))
    cbl[0, 262:390] = 1.0
    return wb.astype(ml_dtypes.bfloat16), cbl


def _shard_inputs(x, Wq, bq, Wk, bk, Wv, bv, gamma):
    """Host-side prep: one input map per core."""
    import ml_dtypes

    xf = np.ascontiguousarray(x, dtype=np.float32).reshape(B, N, CH)
    wb, cbl = _pack_consts(Wq, bq, Wk, bk, Wv, bv, gamma)

    in_maps = []
    for c in range(N_CORES):
        b, h = divmod(c, 2)
        own = slice(h * NQ, (h + 1) * NQ)
        other = slice((1 - h) * NQ, (2 - h) * NQ)
        xT_b = xf[b].T  # [CH, N]
        xT_roll = np.ascontiguousarray(
            np.concatenate([xT_b[:, own], xT_b[:, other]], axis=1)
        ).astype(ml_dtypes.bfloat16)
        in_maps.append(
            {
                "xT": xT_roll,
                "xres": np.ascontiguousarray(xf[b, own]),
                "wblob": wb,
                "cblob": cbl,
            }
        )
    return in_maps


def kernel(x, Wq, bq, Wk, bk, Wv, bv, gamma):
    from concourse.bass_utils import run_bass_kernel_spmd

    nc = _build()
    in_maps = _shard_inputs(x, Wq, bq, Wk, bk, Wv, bv, gamma)
    res = run_bass_kernel_spmd(nc, in_maps, core_ids=list(range(N_CORES)))
    out = np.empty((B, N, CH), np.float32)
    for c in range(N_CORES):
        b, h = divmod(c, 2)
        out[b, h * NQ : (h + 1) * NQ, :] = res.results[c]["y"]
    return out.reshape(x.shape)

